# revision 65
# baseline (speedup 1.0000x reference)
"""DSVF kernel for trn2: biquad SVF via FFT overlap-add in the reference
== exact causal 64-tap FIR (poles |z|=0.426 -> h decays below fp32 eps
by tap ~32).  Implemented as Toeplitz matmuls on TensorE.

v2 layout (vs v1's interleaved-output + out-transposes): the matmul
uses the TRANSPOSED DATA as the stationary operand and the Toeplitz
pair as the moving operand, so the output lands directly in natural
layout -- the 16 out-transposes per row and their PSUM->SBUF copies
disappear.  All PE ops run in f32r (single-pass tf32-like, ~1.5e-4
matmul rel err, measured 2.6e-4 end-to-end on v1; gate is 2e-2).

Math: per batch row, x natural nat[p, 2048] (p = 128 partitions of
2048).  Chunk c = 16p + j covers x[2048p + 128j .. +128).  In-transpose
of natural 128-block j gives xt[q, 128j + p] = chunk(16p+j)[q].
For output natural block b (cols [128b, 128b+128) of every partition):
  stationary xt[:, 128b:128b+128]  (chunk(16m+b) in column m)
  A-move:  out[m, s] += sum_q h[s-q]     x[2048m + 128b + q]   (block b)
  B-move:  out[m, s] += sum_q h[s+128-q] x[2048m + 128b + q]   (block b+1)
Block b's PSUM region accumulates B (from stationary b-1, start=1)
then A (from stationary b, start=0): no vector adds needed.
Seam: block 0's B-contribution comes from chunks 16m-1 (end of the
previous partition row) = block-15 columns shifted by one partition,
realized by a stationary window over [1920:2048) with a permanent
zero-guard column at 1920 (block 15 is stored shifted to 1921..2049).
Block 0 accumulates A(0) at row start and seam-B at row end in a
dedicated PSUM region.

Raw bass (not Tile): PE matmul ISA structs only fit ONE attached sync
wait, which Tile's auto-assigned multi-waits violate.

Engine plan per row r:
  SP  : in-DMA nat[r%2]                   (waits transposes r-2 done)
  PE  : 16 transposes -> px[g%3] (4 groups), then 32 matmuls
        (pairs per stationary j: A(j), B(j+1); j=15: A(15), seam)
  DVE : 4 copies px->xt[r%2]
  ACT : 5 copies pyr/pyz->nato (batches {1-4}{5-8}{9-12}{13-15}{0}),
        out-DMA row
"""

import os
import numpy as np

BATCH = 64
L = 262144
N_CORES = 8
ROWS = BATCH // N_CORES  # 8 rows per core
P = 128
M = L // P  # 2048 columns per row in natural SBUF layout
NBLK = M // P  # 16 blocks per row
K_TAPS = 64
# "bf16": bf16 Toeplitz matmuls (fast path, ~3.8e-3 rel err, gate 2e-2)
# "f32r": single-pass tf32-like matmuls (~2.6e-4 rel err)
# "f32" : two-pass fp32 (~3e-7 rel err)
MM_MODE = os.environ.get("DSVF_MM_MODE", "bf16")
TRACE = os.environ.get("DSVF_TRACE", "0") == "1"

_cache = {}


def _taps(g_param, R_param, m_hp, m_bp, m_lp):
    """64-tap impulse response of the biquad, float64 host math."""
    g = np.tan(np.pi * (1.0 / (1.0 + np.exp(-np.float64(g_param)))) / 2.0)
    R = np.log1p(np.exp(np.float64(R_param)))
    g2 = g * g
    b = [g2 * m_lp + g * m_bp + m_hp,
         2 * g2 * m_lp - 2 * m_hp,
         g2 * m_lp - g * m_bp + m_hp]
    a = [g2 + 2 * R * g + 1, 2 * g2 - 2, g2 - 2 * R * g + 1]
    h = np.zeros(K_TAPS, np.float64)
    for n in range(K_TAPS):
        acc = 0.0
        if n < 3:
            acc += b[n]
        if n >= 1:
            acc -= a[1] * h[n - 1]
        if n >= 2:
            acc -= a[2] * h[n - 2]
        h[n] = acc / a[0]
    return h


def _toeplitz_mats(h):
    A = np.zeros((P, P), np.float32)  # A[q, s] = h[s-q]
    B = np.zeros((P, P), np.float32)  # B[q, s] = h[s-q+128]
    for q in range(P):
        for m in range(P):
            d = m - q
            if 0 <= d < K_TAPS:
                A[q, m] = h[d]
            d2 = m - q + P
            if 0 < d2 < K_TAPS:
                B[q, m] = h[d2]
    return A, B


def _build(mm_mode):
    import concourse.bass as bass
    import concourse.mybir as mybir
    from contextlib import ExitStack

    f32 = mybir.dt.float32
    f32r = mybir.dt.float32r
    bf16 = mybir.dt.bfloat16
    rmode = mm_mode == "f32r"
    bmode = mm_mode == "bf16"
    # bf16 mode: rows 1+ arrive via gpsimd casting DMA (bf16 nat, bf16
    # transposes); the casting DMA runs at ~half read bandwidth, so the
    # latency-critical row 0 arrives as plain f32r (full rate) and runs
    # f32r transposes instead.  The px->xt DVE copy normalizes both to
    # the bf16 stationary.
    # f32r mode: everything f32r (nat declared f32r, DMA'd via f32r APs
    # to satisfy the verifier); the px->xt copy does the true rounding.
    dt_px = f32r if (rmode or bmode) else f32
    dt_nat = bf16 if bmode else (f32r if rmode else f32)
    dt_xt = bf16 if bmode else (f32r if rmode else f32)

    nc = bass.Bass()
    x = nc.declare_dram_parameter("x", [ROWS, L], f32, isOutput=False)
    tid = nc.declare_dram_parameter("tid", [P, P], f32, isOutput=False)
    tab = nc.declare_dram_parameter("tab", [P, 2 * P], f32, isOutput=False)
    y = nc.declare_dram_parameter("y", [ROWS, L], f32, isOutput=True)

    xv = x.rearrange("r (p m) -> r p m", p=P)
    yv = y.rearrange("r (p m) -> r p m", p=P)

    def xb(ap):
        # f32-bit view of f32r storage (px reads)
        return ap.bitcast(f32) if (rmode or bmode) else ap

    def rb(ap):
        # f32r view of f32 dram x
        return ap.bitcast(f32r) if (rmode or bmode) else ap

    with ExitStack() as st:
        ident_f = st.enter_context(nc.sbuf_tensor("ident_f", [P, P], f32))
        tab_f = st.enter_context(nc.sbuf_tensor("tab_f", [P, 2 * P], f32))
        identb = None
        if bmode:
            tabr = st.enter_context(nc.sbuf_tensor("tab_b", [P, 2 * P], bf16))
            identr = st.enter_context(nc.sbuf_tensor("ident_rr", [P, P], f32r))
            identb = st.enter_context(nc.sbuf_tensor("ident_b", [P, P], bf16))
        elif rmode:
            tabr = st.enter_context(nc.sbuf_tensor("tab_r", [P, 2 * P], f32r))
            identr = st.enter_context(nc.sbuf_tensor("ident_rr", [P, P], f32r))
        else:
            tabr, identr = tab_f, ident_f
        # one nat buffer per row: input prefetches with no pipeline
        # coupling.  The latency-critical rows 0..FRONT-1 are f32r
        # (plain full-rate DMA); later rows are bf16 via the casting
        # DMA (half read rate, but they have slack).
        FRONT = 3
        nat = [st.enter_context(nc.sbuf_tensor(
            f"nat{i}", [P, M], dt_px if i < FRONT else dt_nat))
            for i in range(ROWS)]
        # xt: blocks 0..14 at [128j, 128j+128); permanent zero-guard col
        # at 1920; block 15 shifted to 1921..2049.
        xt = [st.enter_context(nc.sbuf_tensor(f"xt{i}", [P, M + 1], dt_xt))
              for i in range(2)]
        # one nato buffer per row: no write-after-read coupling with the
        # out-DMA queue (its backlog must not stall the copy engine)
        nato = [st.enter_context(nc.sbuf_tensor(f"nato{i}", [P, M], f32))
                for i in range(ROWS)]
        # px declared f32 (one bank each); rows view it as f32r (row 0)
        # or bf16 (rows 1+, bmode) to match their transpose dtype
        px = [st.enter_context(nc.psum_tensor(f"px{i}", [P, 512], f32))
              for i in range(3)]

        def px_w(i, r):
            # transpose-output view of px[i] for row r
            if r < FRONT or rmode:
                return px[i][:].bitcast(f32r) if (rmode or bmode) else px[i][:]
            return px[i][:].bitcast(bf16) if bmode else px[i][:]

        def px_r(i, r):
            # DVE-read view (f32 bits for converting copies; bf16 direct)
            if bmode and r >= FRONT:
                return px[i][:].bitcast(bf16)[:, 0:512]
            return px[i][:]

        def tp_ident(r):
            return identb[:] if (bmode and r >= FRONT) else identr[:]
        # pyr: 12-slot ring (128 cols each) for blocks 1..15, slot (b-1)%12
        pyr = [st.enter_context(nc.psum_tensor(f"pyr{i}", [P, 512], f32))
               for i in range(3)]
        # pyz: block 0 region (A at row start + seam-B at row end),
        # double-buffered by row parity
        pyz = st.enter_context(nc.psum_tensor("pyz", [P, 2 * P], f32))

        dCst = st.enter_context(nc.semaphore("dCst"))
        sInit = st.enter_context(nc.semaphore("sInit"))
        # per-transfer DMA-completion sems (16 incs each)
        dC = [st.enter_context(nc.semaphore(f"dC{g}")) for g in range(2)]
        dIn = [st.enter_context(nc.semaphore(f"dIn{r}"))
               for r in range(1, ROWS)]
        dOutAll = st.enter_context(nc.semaphore("dOutAll"))
        sTp = st.enter_context(nc.semaphore("sTp"))  # +1 per transpose group
        sXt = st.enter_context(nc.semaphore("sXt"))  # +1 per px->xt copy group
        sMm = st.enter_context(nc.semaphore("sMm"))  # +1 per matmul
        sCp = st.enter_context(nc.semaphore("sCp"))  # +1 per ACT copy batch

        # skip gpsimd's expensive dge_drain at block exit: its ring only
        # carries input rows, which are fully consumed before the
        # sem-gated output DMAs (waited on explicitly) can finish
        blk = st.enter_context(nc.Block(no_gpsimd_drain=True))

        @blk.gpsimd
        def _(gp):
            # the gpsimd DGE ring runs at roughly half bandwidth, so it
            # only carries the consts and the late (slack-rich) rows --
            # which also lets them be casting f32->bf16 transfers.
            gp.dma_start(out=ident_f[:], in_=tid[:]).then_inc(dCst, 16)
            gp.dma_start(out=tab_f[:], in_=tab[:]).then_inc(dCst, 16)
            gp.wait_ge(dC[1], 16)  # let row 0 have the HBM first
            for r in range(FRONT, ROWS):
                gp.dma_start(out=nat[r][:],
                             in_=xv[r] if bmode else rb(xv[r])
                             ).then_inc(dIn[r - 1], 16)

        def slot(b):
            s = (b - 1) % 12
            return pyr[s // 4][:, (s % 4) * P:(s % 4 + 1) * P]

        def pyzr(r):
            return pyz[:, (r % 2) * P:(r % 2 + 1) * P]

        @blk.tensor
        def _(pe):
            pe.wait_ge(dCst, 32)
            pe.wait_ge(sInit, 1)
            for r in range(ROWS):
                # in-transposes: group g covers blocks 4g..4g+3
                for g in range(4):
                    gg = 4 * r + g
                    if r == 0:
                        if g % 2 == 0:
                            pe.wait_ge(dC[g // 2], 16)
                    elif g == 0:
                        pe.wait_ge(dIn[r - 1], 16)
                    if gg >= 3:
                        pe.wait_ge(sXt, gg - 2)  # px[gg%3] freed
                    dst = px_w(gg % 3, r)
                    for jj in range(4):
                        j = 4 * g + jj
                        ins = pe.transpose(
                            dst[:, jj * P:(jj + 1) * P],
                            nat[r][:, j * P:(j + 1) * P],
                            tp_ident(r))
                    ins.then_inc(sTp, 1)
                # matmul pairs per stationary j: A(j) then B(j+1)/seam.
                # mm idx within row: A(b)=2b, B(b)=2b-1, seam=31.
                for j in range(NBLK):
                    pe.wait_ge(sXt, 4 * r + j // 4 + 1)
                    if j == 0:
                        # ring slots 0..2 freed by row r-1's copy batch 4
                        # (their last writers were blocks 13..15); pyz
                        # region r%2 freed by row r-2's copy batch 5.
                        if r >= 1:
                            pe.wait_ge(sCp, 5 * r - 1)
                        lhs = xt[r % 2][:, 0:P]
                        ins = pe.matmul(pyzr(r), lhs, tabr[:, 0:P],
                                        start=True, stop=False,
                                        skip_group_check=True)
                        ins.then_inc(sMm, 1)  # A(0)
                        ins = pe.matmul(slot(1), lhs, tabr[:, P:2 * P],
                                        start=True, stop=False,
                                        skip_group_check=True)
                        ins.then_inc(sMm, 1)  # B(1)
                    elif j < NBLK - 1:
                        # j==12: B(13)->slot 0, freed by THIS row's copy
                        # batch 1 (blocks 1..4 -> slots 0..3)
                        if j == 12:
                            pe.wait_ge(sCp, 5 * r + 1)
                        lhs = xt[r % 2][:, j * P:(j + 1) * P]
                        ins = pe.matmul(slot(j), lhs, tabr[:, 0:P],
                                        start=False, stop=True,
                                        skip_group_check=True)
                        ins.then_inc(sMm, 1)  # A(j)
                        ins = pe.matmul(slot(j + 1), lhs, tabr[:, P:2 * P],
                                        start=True, stop=False,
                                        skip_group_check=True)
                        ins.then_inc(sMm, 1)  # B(j+1)
                    else:
                        # j==15: A(15) from shifted block 15, then seam-B
                        ins = pe.matmul(slot(15), xt[r % 2][:, 1921:2049],
                                        tabr[:, 0:P],
                                        start=False, stop=True,
                                        skip_group_check=True)
                        ins.then_inc(sMm, 1)  # A(15)
                        ins = pe.matmul(pyzr(r), xt[r % 2][:, 1920:2048],
                                        tabr[:, P:2 * P],
                                        start=False, stop=True,
                                        skip_group_check=True)
                        ins.then_inc(sMm, 1)  # seam -> block 0

        @blk.vector
        def _(dve):
            dve.wait_ge(dCst, 32)
            if rmode or bmode:
                dve.tensor_copy(tabr[:], tab_f[:])
                dve.tensor_copy(identr[:], ident_f[:])
                if bmode:
                    dve.tensor_copy(identb[:], ident_f[:])
            # permanent seam guard columns
            gv = (lambda ap: ap.bitcast(f32)) if rmode else (lambda ap: ap)
            dve.memset(gv(xt[0][:, 1920:1921]), 0.0)
            ins = dve.memset(gv(xt[1][:, 1920:1921]), 0.0)
            ins.then_inc(sInit, 1)

            for r in range(ROWS):
                if r >= 2:
                    dve.wait_ge(sMm, 32 * (r - 1))  # xt[r%2] still read
                for g in range(4):
                    gg = 4 * r + g
                    dve.wait_ge(sTp, gg + 1)
                    # row 0 / f32r: px holds f32 bits, this copy IS the
                    # rounding into the bf16/f32r stationary; bf16 rows
                    # are a plain bf16 bit copy
                    pxs = px_r(gg % 3, r)
                    if g == 3:
                        dve.tensor_copy(xt[r % 2][:, 1536:1920],
                                        pxs[:, 0:384])
                        ins = dve.tensor_copy(xt[r % 2][:, 1921:2049],
                                              pxs[:, 384:512])
                    else:
                        ins = dve.tensor_copy(
                            xt[r % 2][:, g * 512:(g + 1) * 512], pxs)
                    ins.then_inc(sXt, 1)

        @blk.scalar
        def _(act):
            for r in range(ROWS):
                # copy batches: {1-4}{5-8}{9-12}{13-15}{0}
                batches = [
                    (32 * r + 9, pyr[0][:, 0:512], 1, 4),
                    (32 * r + 17, pyr[1][:, 0:512], 5, 4),
                    (32 * r + 25, pyr[2][:, 0:512], 9, 4),
                    (32 * r + 31, pyr[0][:, 0:384], 13, 3),
                    (32 * r + 32, pyzr(r), 0, 1),
                ]
                for (mmw, src, b0, nb) in batches:
                    act.wait_ge(sMm, mmw)
                    ins = act.copy(out=nato[r][:, b0 * P:(b0 + nb) * P],
                                   in_=src)
                    ins.then_inc(sCp, 1)

        @blk.sync
        def _(sp):
            # the fast SP ring carries the latency-critical input rows
            # 0..FRONT-1, then the out-DMA triggers.  The DMA engines
            # drain ALL in-flight transfers together with no FIFO
            # priority, so the critical-path transfers are explicitly
            # serialized: row-0 halves first (alone), then rows 1, 2.
            for g in range(2):
                sp.dma_start(out=nat[0][:, g * 1024:(g + 1) * 1024],
                             in_=rb(xv[0][:, g * 1024:(g + 1) * 1024])
                             ).then_inc(dC[g], 16)
                sp.wait_ge(dC[g], 16)
            for r in range(1, FRONT):
                sp.dma_start(out=nat[r][:],
                             in_=rb(xv[r])).then_inc(dIn[r - 1], 16)
                if r < FRONT - 1:
                    sp.wait_ge(dIn[r - 1], 16)
            for r in range(ROWS):
                last = r == ROWS - 1
                if last:
                    # per-batch stores to shorten the tail
                    for k, (b0, nb) in enumerate([(1, 4), (5, 4), (9, 4),
                                                  (13, 3)]):
                        sp.wait_ge(sCp, 5 * r + k + 1)
                        sp.dma_start(out=yv[r][:, b0 * P:(b0 + nb) * P],
                                     in_=nato[r][:, b0 * P:(b0 + nb) * P]
                                     ).then_inc(dOutAll, 16)
                else:
                    sp.wait_ge(sCp, 5 * r + 4)
                    sp.dma_start(out=yv[r][:, P:M],
                                 in_=nato[r][:, P:M]
                                 ).then_inc(dOutAll, 16)
                sp.wait_ge(sCp, 5 * r + 5)
                sp.dma_start(out=yv[r][:, 0:P],
                             in_=nato[r][:, 0:P]
                             ).then_inc(dOutAll, 16)
            # drain: rows 0..6 have 2 transfers each, row 7 has 5
            sp.wait_ge(dOutAll, 16 * (2 * (ROWS - 1) + 5))

    return nc


def _get_nc():
    key = MM_MODE
    if key not in _cache:
        _cache[key] = _build(MM_MODE)
    return _cache[key]


def kernel(**inputs):
    from concourse.bass_utils import run_bass_kernel_spmd

    x = np.ascontiguousarray(np.asarray(inputs["x"], dtype=np.float32))
    assert x.shape == (BATCH, L), x.shape
    h = _taps(float(np.asarray(inputs["g_param"]).reshape(-1)[0]),
              float(np.asarray(inputs["R_param"]).reshape(-1)[0]),
              float(np.asarray(inputs["m_hp"]).reshape(-1)[0]),
              float(np.asarray(inputs["m_bp"]).reshape(-1)[0]),
              float(np.asarray(inputs["m_lp"]).reshape(-1)[0]))
    A, B = _toeplitz_mats(h)
    ident = np.eye(P, dtype=np.float32)
    common = {"tid": ident, "tab": np.concatenate([A, B], axis=1)}

    nc = _get_nc()
    core_ids = list(range(N_CORES))
    in_maps = [
        {"x": x[i * ROWS:(i + 1) * ROWS], **common}
        for i in range(N_CORES)
    ]
    kwargs = {}
    if TRACE:
        kwargs["tmpdir"] = os.environ.get("DSVF_TRACE_DIR") or None
    res = run_bass_kernel_spmd(nc, in_maps, core_ids, trace=TRACE, **kwargs)
    if TRACE:
        kernel.last_exec_time_ns = res.exec_time_ns
        kernel.last_results = res
    out = np.concatenate([res.results[i]["y"] for i in range(N_CORES)], axis=0)
    return out.astype(np.float32, copy=False)


kernel.last_exec_time_ns = None
